# revision 1
# baseline (speedup 1.0000x reference)
"""Deformable Conv2d (3x3, stride 1, pad 1) + BatchNorm (batch stats) + ReLU
on 8 Trainium2 NeuronCores (Bass/Tile).

Sharding: core i handles sample n = i // 2, row half h0 = (i % 2) * 48,
computing all 256 output channels for its 48x96 half plane.  BatchNorm
statistics are AllReduced across all 8 cores.

Per-core pipeline:
  1. offset conv (18 ch) as PSUM-accumulated shifted matmuls (fp32r)
  2. PE transposes into layout B: partition p = g*16+q, col s  <->
     position m = g*576 + s*16 + q   (m = h_local*96 + w)
  3. DVE index/weight math; floor via int-convert with round-mode guard;
     corners clipped into a 98x98 zero-padded plane (padding replaces all
     out-of-bounds masking exactly)
  4. wrapped int16 index tiles for ap_gather (its per-16-partition layout)
     and bilinear corner-weight rows, built via 8+8 g-blocked DMA folds
     through DRAM
  5. GPSIMD ap_gather (4 corners x 9 taps x 2 cblocks) + DVE blend
  6. main conv: PSUM accumulation over (tap, cblock) of fp32r matmuls
  7. BN stats (ACT accum) -> AllReduce -> scale/bias -> fused Relu apply
"""

import sys

if "/opt/trn_rl_repo" not in sys.path:
    sys.path.insert(0, "/opt/trn_rl_repo")

import numpy as np

# ---------------- problem constants (hardcoded) ----------------
N, C, H, W = 4, 256, 96, 96
O = 256
K = 9                      # taps
CB = 2                     # channel blocks of 128
HP = 98                    # padded plane side
PLANE = HP * HP            # 9604
ROWS = 48                  # output rows per core
M = ROWS * W               # 4608 positions per core
SEG = M // 8               # 576
SW = M // 16               # 288 wrapped columns per tap-corner
NT = 2                     # halves (a half = 4 g-groups)
MS = M // NT               # 1152
GPT = 8 // NT              # g-groups per strip
SWT = SW // NT             # 72 wrapped cols per strip
EPS = 1e-5
NCORES = 8
TC = 36                    # tap-corner pairs; t = cr*9 + k


def _body(tcx, aps, num_devices):
    import concourse.mybir as mybir

    nc = tcx.nc
    dt = mybir.dt
    f32, f32r, i32, i16 = dt.float32, dt.float32r, dt.int32, dt.int16
    bf16 = dt.bfloat16
    AF = mybir.ActivationFunctionType
    ALU = mybir.AluOpType

    x_cb = aps["x_cb"]           # (CB, 128, 96, 96) f32
    x_strip_in = aps["x_strip"]  # (CB, 128, 50, 98) f32 rows h0-1..h0+48, padded
    woff_in = aps["w_off_t"]     # (K, CB, 128, 18) f32
    wdcn_in = aps["w_dcn_t"]     # (K, CB, 128, O) f32
    gamma_in = aps["gamma2"]     # (128, CB) f32
    beta_in = aps["beta2"]       # (128, CB) f32
    p0_in = aps["p0"]            # (128, 648) f32 : grid + tap + b_off + 16, layout B
    y_out = aps["y_out"]         # (CB, 128, M) f32

    # ---------------- persistent tiles ----------------
    with tcx.tile_pool(name="pers", bufs=1) as pers, \
         tcx.tile_pool(name="dram", bufs=1, space="DRAM") as dram:
        xpad = [pers.tile([128, PLANE], f32, tag=f"xpad{cb}", name=f"xpad{cb}") for cb in range(CB)]
        wdcn_sb = pers.tile([128, K * CB * O], f32r, tag="wdcn")
        bnsb16 = pers.tile([128, 16], f32, tag="bnsb16")
        gb_sb = bnsb16[:, 12:16]
        idx16 = pers.tile([128, TC * SW], i16, tag="idx16")
        bnsb = bnsb16[:, 0:8]
        stats = bnsb16[:, 8:12]

        idx_bounce = dram.tile([16, TC * SW], i16, tag="idxb")
        wgt_bounce = dram.tile([TC, M], bf16, tag="wgtb")
        cc_in = dram.tile([128, 4], f32, tag="ccin")
        cc_out = dram.tile([128, 4], f32, tag="ccout")

        for cb in range(CB):
            nc.vector.memset(xpad[cb][:], 0.0)
            nc.sync.dma_start(
                xpad[cb][:].rearrange("p (h w) -> p h w", h=HP)[:, 1:97, 1:97],
                x_cb[cb],
            )
        nc.sync.dma_start(gb_sb[:, 0:CB], gamma_in)
        nc.sync.dma_start(gb_sb[:, CB : 2 * CB], beta_in)

        # ---------------- phase 1: offset conv ----------------
        emid_cm = tcx.tile_pool(name="emid", bufs=1)
        emid = emid_cm.__enter__()
        woff_sb = emid.tile([128, K * CB * 18], f32r, tag="woff", name="woffr")
        dydx = emid.tile([128, 36 * 18], f32, tag="dydx", name="dydx")
        with tcx.tile_pool(name="early1", bufs=1) as early1, \
             tcx.tile_pool(name="ps_off", bufs=2, space="PSUM") as ps_off:
            off_sb = early1.tile([32, M], f32, tag="off")
            nc.vector.memset(off_sb[:], 0.0)
            # stage f32 weights, round to f32r via DVE (fp32r matmul contract)
            wstage = early1.tile([128, K * CB * 18], f32, tag="wstage", name="wst")
            nc.sync.dma_start(wstage[:], woff_in.rearrange("k c p m -> p (k c) m"))
            nc.vector.tensor_copy(woff_sb[:], wstage[:])
            wdv_in = wdcn_in.rearrange("k c p m -> p (k c) m")
            for cbh in range(CB):
                dstage = early1.tile([128, K * O], f32, tag="dstage",
                                     name=f"dstg{cbh}")
                nc.sync.dma_start(
                    dstage[:],
                    wdv_in[:, cbh : K * CB : CB, :],
                )
                nc.vector.tensor_copy(
                    wdcn_sb[:].rearrange("p (k c m) -> p k c m", k=K, c=CB)[
                        :, :, cbh, :
                    ],
                    dstage[:].rearrange("p (k m) -> p k m", k=K),
                )
            xs = [early1.tile([128, 26 * HP], f32, tag=f"xs{cb}", name=f"xs{cb}") for cb in range(CB)]
            xsr = [early1.tile([128, 26 * HP], f32r, tag=f"xsr{cb}", name=f"xsr{cb}") for cb in range(CB)]
            woff_v = woff_sb[:].rearrange("p (k c m) -> p k c m", k=K, c=CB)

            for half in range(2):
                rbase = half * 24
                for cb in range(CB):
                    nc.sync.dma_start(
                        xs[cb][:],
                        x_strip_in[cb][:, rbase : rbase + 26, :].rearrange(
                            "p h w -> p (h w)"
                        ),
                    )
                    nc.vector.tensor_copy(xsr[cb][:], xs[cb][:])
                xsv = [
                    xsr[cb][:].rearrange("p (h w) -> p h w", h=26)
                    for cb in range(CB)
                ]
                for chunk in range(6):        # 6 chunks of 4 rows = 384 cols
                    r0 = chunk * 4
                    po = ps_off.tile([18, 384], f32, tag="po")
                    li = 0
                    for k in range(K):
                        ky, kx = k // 3 - 1, k % 3 - 1
                        for cb in range(CB):
                            rhs = xsv[cb][
                                :, r0 + ky + 1 : r0 + ky + 5, kx + 1 : kx + 97
                            ]
                            nc.tensor.matmul(
                                po[:],
                                woff_v[:, k, cb],
                                rhs,
                                start=(li == 0),
                                stop=(li == 2 * K - 1),
                            )
                            li += 1
                    g0 = (rbase + r0) * 96
                    nc.scalar.copy(off_sb[0:18, g0 : g0 + 384], po[:])

            # ------------ phase 2: DVE 32x32 block transpose to layout B --
            # offT (stream transpose) viewed (32, 144, 32):
            #   offT[m % 32, m // 32, tap] = off[tap, m]
            # layout B: dydx[g*16+q, s, tap] = off[tap, g*576 + s*16 + q]
            #   = offT[(s%2)*16 + q, g*18 + s//2, tap]
            offT = early1.tile([32, M], f32, tag="offT")
            nc.vector.transpose(offT[:], off_sb[:])
            offT_v = offT[:].rearrange("p (t s) -> p t s", s=32)
            dydx_v3 = dydx[:].rearrange("p (s t) -> p s t", t=18)
            for g in range(8):
                for s1 in range(2):
                    nc.sync.dma_start(
                        dydx_v3[g * 16 : (g + 1) * 16, s1 : 36 : 2, :],
                        offT_v[s1 * 16 : (s1 + 1) * 16,
                               g * 18 : (g + 1) * 18, 0:18],
                    )

        # ---------------- phase 3: index & weight math ----------------
        with tcx.tile_pool(name="early2", bufs=1) as early2:
            p0_sb = early2.tile([128, 648], f32, tag="p0")
            nc.sync.dma_start(p0_sb[:], p0_in)
            pp = early2.tile([128, 648], f32, tag="pp")
            tf = early2.tile([128, 648], f32, tag="tf")
            ti = early2.tile([128, 648], i32, tag="ti")
            wfr = early2.tile([128, 648], f32, tag="wfr")
            ca = early2.tile([128, 648], f32, tag="ca")
            cbt = early2.tile([128, 648], f32, tag="cbt")
            sc1 = early2.tile([128, 324], f32, tag="sc1")
            sc2 = early2.tile([128, 324], f32, tag="sc2")
            idxf = early2.tile([128, 4 * 324], f32, tag="idxf")
            idxi = early2.tile([128, 4 * 324], i32, tag="idxi")
            idxm16 = early2.tile([128, TC * 36], i16, tag="idxm16")
            wgt_b = early2.tile([128, 4 * 324], bf16, tag="wgtb")

            nc.vector.tensor_add(pp[:], dydx[:], p0_sb[:])   # P = py|px + 16
            nc.vector.tensor_copy(ti[:], pp[:])
            nc.vector.tensor_copy(tf[:], ti[:])
            nc.vector.tensor_tensor(wfr[:], tf[:], pp[:], ALU.is_gt)
            nc.vector.tensor_sub(tf[:], tf[:], wfr[:])       # fl = floor(P)
            nc.vector.tensor_sub(wfr[:], pp[:], tf[:])       # frac
            # corner pad-coords: A = clip(fl-15, 0, 97); B = clip(fl-14, 0, 97)
            nc.vector.tensor_scalar(ca[:], tf[:], 15.0, 0.0, ALU.subtract, ALU.max)
            nc.vector.tensor_scalar_min(ca[:], ca[:], 97.0)
            nc.vector.tensor_scalar(cbt[:], tf[:], 14.0, 0.0, ALU.subtract, ALU.max)
            nc.vector.tensor_scalar_min(cbt[:], cbt[:], 97.0)

            def yx(t, d):  # (128, 36, 9) strided view; d=0 -> y cols, 1 -> x
                return t[:].rearrange("p (s k d) -> p s k d", k=K, d=2)[
                    :, :, :, d
                ]

            idxf_v = idxf[:].rearrange("p (cr k s) -> p cr k s", cr=4, k=K)
            wgt_v = wgt_b[:].rearrange("p (cr k s) -> p cr k s", cr=4, k=K)

            def okv(cr):   # write view, enumeration (s, k)
                return idxf_v[:, cr].transpose([0, 2, 1])

            def wkv(cr):
                return wgt_v[:, cr].transpose([0, 2, 1])

            sc1v = sc1[:].rearrange("p (s k) -> p s k", k=K)
            sc2v = sc2[:].rearrange("p (s k) -> p s k", k=K)
            nc.vector.tensor_scalar_mul(sc1v, yx(ca, 0), 98.0)
            nc.vector.tensor_scalar_mul(sc2v, yx(cbt, 0), 98.0)
            nc.vector.tensor_add(okv(0), sc1v, yx(ca, 1))    # (y0, x0)
            nc.vector.tensor_add(okv(1), sc1v, yx(cbt, 1))   # (y0, x1)
            nc.vector.tensor_add(okv(2), sc2v, yx(ca, 1))    # (y1, x0)
            nc.vector.tensor_add(okv(3), sc2v, yx(cbt, 1))   # (y1, x1)
            nc.vector.tensor_copy(idxi[:], idxf[:])
            nc.vector.tensor_copy(idxm16[:], idxi[:])

            wa = pp  # reuse
            nc.vector.tensor_scalar(wa[:], wfr[:], -1.0, 1.0, ALU.mult, ALU.add)
            nc.vector.tensor_mul(wkv(0), yx(wa, 0), yx(wa, 1))
            nc.vector.tensor_mul(wkv(1), yx(wa, 0), yx(wfr, 1))
            nc.vector.tensor_mul(wkv(2), yx(wfr, 0), yx(wa, 1))
            nc.vector.tensor_mul(wkv(3), yx(wfr, 0), yx(wfr, 1))

            # ---- phase 4: g-blocked folds through DRAM ----
            idxm_v = idxm16[:].rearrange("p (t s) -> p t s", t=TC)
            ixb_v = idx_bounce[:].rearrange("q (t s) -> q t s", t=TC)
            wgb_v = wgt_bounce[:].rearrange("t (p s) -> t p s", p=128)
            wgm_v = wgt_b[:].rearrange("p (t s) -> p t s", t=TC)
            for g in range(8):
                nc.scalar.dma_start(
                    ixb_v[:, :, g * 36 : (g + 1) * 36],
                    idxm_v[g * 16 : (g + 1) * 16, :, :],
                )
                nc.scalar.dma_start(
                    wgb_v[:, g * 16 : (g + 1) * 16, :].transpose([1, 0, 2]),
                    wgm_v[g * 16 : (g + 1) * 16, :, :],
                )
            for g2 in range(8):
                nc.sync.dma_start(
                    idx16[g2 * 16 : (g2 + 1) * 16, :], idx_bounce[:]
                )

        emid_cm.__exit__(None, None, None)
        # ---------------- phase 5+6: gather / blend / matmul ----------------
        # ap_gather streams its source plane, so fewer+bigger gathers win:
        # half-plane gathers (num_idxs 2304), tap-outer loop, y accumulated
        # in SBUF (PSUM stays at 4 banks via single-shot matmuls + DVE adds).
        with tcx.tile_pool(name="gpool", bufs=2) as gpool, \
             tcx.tile_pool(name="bpool", bufs=1) as bpool, \
             tcx.tile_pool(name="spool", bufs=1) as spool, \
             tcx.tile_pool(name="wpool", bufs=2) as wpool, \
             tcx.tile_pool(name="ypool", bufs=1) as ypool, \
             tcx.tile_pool(name="ps_y", bufs=4, space="PSUM") as ps_y:

            nc.vector.memset(stats, 0.0)
            y_acc = [ypool.tile([128, M], f32, tag=f"yacc{mt}", name=f"yacc{mt}")
                     for mt in range(2)]
            for mt in range(2):
                nc.vector.memset(y_acc[mt][:], 0.0)
            wdcn_v = wdcn_sb[:].rearrange("p (k c m) -> p k c m", k=K, c=CB)
            wgb_r = wgt_bounce[:]
            CHUNKS = [(0, 512), (512, 512), (1024, 512), (1536, 512), (2048, 256)]

            for hp in range(NT):
                for k in range(K):
                    wr4 = []
                    for cr in range(4):
                        tcid = cr * 9 + k
                        wr = wpool.tile([128, MS], bf16, tag="wr",
                                        name=f"wr{hp}{tcid}")
                        nc.scalar.dma_start(
                            wr[:].unsqueeze(1),
                            wgb_r[
                                tcid : tcid + 1, hp * MS : (hp + 1) * MS
                            ].unsqueeze(0).to_broadcast((128, 1, MS)),
                        )
                        wr4.append(wr)

                    def mvw(t):  # m-contiguous tile -> (p, g, s, q) view
                        return t.rearrange("p (g s q) -> p g s q", g=GPT, q=16)

                    def wv(cr):  # B-dump-ordered row -> (p, g, s, q) m-order
                        return wr4[cr][:].rearrange(
                            "p (g q s) -> p g s q", g=GPT, q=16
                        )

                    acc = [bpool.tile([128, MS], bf16, tag=f"acc{cb}",
                                      name=f"ac{hp}{k}{cb}") for cb in range(CB)]
                    stv = [spool.tile([128, MS], f32r, tag=f"s{cb}",
                                      name=f"sv{hp}{k}{cb}") for cb in range(CB)]
                    for cr in range(4):
                        tcid = cr * 9 + k
                        ix = idx16[
                            :, tcid * SW + hp * SWT : tcid * SW + (hp + 1) * SWT
                        ]
                        for cb in range(CB):
                            go = gpool.tile([128, MS], f32, tag="go",
                                            name=f"go{tcid}{cb}")
                            nc.gpsimd.ap_gather(
                                go[:], xpad[cb][:], ix,
                                channels=128, num_elems=PLANE, d=1, num_idxs=MS,
                            )
                            if cr == 0:
                                nc.vector.tensor_mul(
                                    mvw(acc[cb][:]), mvw(go[:]), wv(0)
                                )
                            else:
                                nc.vector.tensor_mul(
                                    mvw(go[:]), mvw(go[:]), wv(cr)
                                )
                                dst = acc[cb][:] if cr < 3 else stv[cb][:]
                                nc.vector.tensor_add(
                                    dst, acc[cb][:], go[:]
                                )
                    for cb in range(CB):
                        stile = stv[cb]
                        for mt in range(2):
                            lhsT = wdcn_v[:, k, cb, mt * 128 : (mt + 1) * 128]
                            for c0, cn in CHUNKS:
                                psy = ps_y.tile([128, 512], f32, tag="psy",
                                                name=f"p{hp}{k}{cb}{mt}{c0}")
                                nc.tensor.matmul(
                                    psy[:, :cn], lhsT,
                                    stile[:, c0 : c0 + cn],
                                    start=True, stop=True,
                                )
                                sl = slice(hp * MS + c0, hp * MS + c0 + cn)
                                nc.vector.tensor_add(
                                    y_acc[mt][:, sl], y_acc[mt][:, sl],
                                    psy[:, :cn],
                                )
            # stats on the fully accumulated y (scratch borrows a gout slot)
            for mt in range(2):
                s_p = bnsb16[:, 4:8]
                for hp in range(2):
                    sl = slice(hp * MS, (hp + 1) * MS)
                    sq = gpool.tile([128, MS], f32, tag="go", name=f"sq{mt}{hp}")
                    nc.vector.tensor_mul(sq[:], y_acc[mt][:, sl], y_acc[mt][:, sl])
                    nc.vector.tensor_reduce(
                        s_p[:, hp : hp + 1], y_acc[mt][:, sl],
                        mybir.AxisListType.X, ALU.add,
                    )
                    nc.vector.tensor_reduce(
                        s_p[:, 2 + hp : 3 + hp], sq[:],
                        mybir.AxisListType.X, ALU.add,
                    )
                nc.vector.tensor_add(stats[:, mt : mt + 1], s_p[:, 0:1],
                                     s_p[:, 1:2])
                nc.vector.tensor_add(stats[:, 2 + mt : 3 + mt], s_p[:, 2:3],
                                     s_p[:, 3:4])

        # ---------------- phase 7: BN reduce + apply ----------------
        with tcx.tile_pool(name="fin", bufs=2) as fin:
            nc.sync.dma_start(cc_in[:], stats)
            if num_devices > 1:
                nc.gpsimd.collective_compute(
                    "AllReduce",
                    mybir.AluOpType.add,
                    replica_groups=[list(range(num_devices))],
                    ins=[cc_in.opt()],
                    outs=[cc_out.opt()],
                )
            else:
                nc.sync.dma_start(cc_out[:], cc_in[:])
            nc.sync.dma_start(stats, cc_out[:])
            cnt = float(NCORES * M)
            nc.vector.tensor_scalar_mul(bnsb[:, 0:2], stats[:, 0:2], 1.0 / cnt)
            nc.vector.tensor_scalar_mul(bnsb[:, 2:4], stats[:, 2:4], 1.0 / cnt)
            nc.vector.tensor_mul(bnsb[:, 6:8], bnsb[:, 0:2], bnsb[:, 0:2])
            nc.vector.tensor_sub(bnsb[:, 2:4], bnsb[:, 2:4], bnsb[:, 6:8])
            nc.vector.tensor_scalar_add(bnsb[:, 2:4], bnsb[:, 2:4], EPS)
            nc.scalar.activation(bnsb[:, 2:4], bnsb[:, 2:4], AF.Sqrt)
            nc.vector.reciprocal(bnsb[:, 2:4], bnsb[:, 2:4])
            nc.vector.tensor_mul(bnsb[:, 4:6], bnsb[:, 2:4], gb_sb[:, 0:CB])
            nc.vector.tensor_mul(bnsb[:, 6:8], bnsb[:, 0:2], bnsb[:, 4:6])
            nc.vector.tensor_sub(
                bnsb[:, 6:8], gb_sb[:, CB : 2 * CB], bnsb[:, 6:8]
            )

            for cb in range(CB):
                for hp in range(2):
                    sl = slice(hp * MS, (hp + 1) * MS)
                    yf = fin.tile([128, MS], f32, tag="yf", name=f"yf{cb}{hp}")
                    nc.scalar.activation(
                        yf[:], y_acc[cb][:, sl], AF.Relu,
                        bias=bnsb[:, 6 + cb : 7 + cb],
                        scale=bnsb[:, 4 + cb : 5 + cb],
                    )
                    nc.sync.dma_start(y_out[cb][:, sl], yf[:])


def build_program(num_devices=NCORES):
    import concourse.mybir as mybir
    import concourse.tile as tile
    from concourse import bacc

    dt = mybir.dt
    nc = bacc.Bacc(
        "TRN2",
        target_bir_lowering=False,
        debug=False,
        enable_asserts=False,
        num_devices=num_devices,
    )
    f32 = dt.float32
    aps = {
        "x_cb": nc.dram_tensor("x_cb", (CB, 128, H, W), f32, kind="ExternalInput").ap(),
        "x_strip": nc.dram_tensor("x_strip", (CB, 128, 50, HP), f32, kind="ExternalInput").ap(),
        "w_off_t": nc.dram_tensor("w_off_t", (K, CB, 128, 18), f32, kind="ExternalInput").ap(),
        "w_dcn_t": nc.dram_tensor("w_dcn_t", (K, CB, 128, O), f32, kind="ExternalInput").ap(),
        "gamma2": nc.dram_tensor("gamma2", (128, CB), f32, kind="ExternalInput").ap(),
        "beta2": nc.dram_tensor("beta2", (128, CB), f32, kind="ExternalInput").ap(),
        "p0": nc.dram_tensor("p0", (128, 648), f32, kind="ExternalInput").ap(),
        "y_out": nc.dram_tensor("y_out", (CB, 128, M), f32, kind="ExternalOutput").ap(),
    }
    import concourse.tile as tile_mod
    with tile_mod.TileContext(nc) as tcx:
        _body(tcx, aps, num_devices)
    nc.compile()
    return nc


# ---------------- host-side input marshalling (numpy only) ----------------

def make_core_inputs(x, w_off, b_off, w_dcn, gamma, beta, core):
    n, half = core // 2, core % 2
    h0 = half * ROWS
    xs = np.ascontiguousarray(np.asarray(x[n], dtype=np.float32))   # (C, H, W)
    x_cbv = xs.reshape(CB, 128, H, W)
    x_strip = np.zeros((CB, 128, 50, HP), np.float32)
    r0, r1 = h0 - 1, h0 + 49
    s0, s1 = max(r0, 0), min(r1, H)
    x_strip[:, :, s0 - r0 : s0 - r0 + (s1 - s0), 1:97] = x_cbv[:, :, s0:s1, :]

    w_off_t = np.ascontiguousarray(
        np.asarray(w_off, np.float32)
        .reshape(18, CB, 128, 3, 3)
        .transpose(3, 4, 1, 2, 0)
        .reshape(K, CB, 128, 18)
    )
    w_dcn_t = np.ascontiguousarray(
        np.asarray(w_dcn, np.float32)
        .reshape(O, CB, 128, K)
        .transpose(3, 1, 2, 0)
    )
    gamma2 = np.ascontiguousarray(np.asarray(gamma, np.float32).reshape(CB, 128).T)
    beta2 = np.ascontiguousarray(np.asarray(beta, np.float32).reshape(CB, 128).T)

    # p0 in layout B: partition p = g*16+q, col (s, t): m = g*576 + s*16 + q
    p = np.arange(128)
    s = np.arange(36)
    m = (p[:, None] // 16) * SEG + s[None, :] * 16 + (p[:, None] % 16)
    hl, wl = m // W, m % W
    ky = np.arange(K) // 3 - 1
    kx = np.arange(K) % 3 - 1
    b2 = np.asarray(b_off, np.float32).reshape(K, 2)
    p0 = np.zeros((128, 36, K, 2), np.float32)
    p0[..., 0] = (h0 + hl)[:, :, None] + ky[None, None, :] + b2[None, None, :, 0] + 16.0
    p0[..., 1] = wl[:, :, None] + kx[None, None, :] + b2[None, None, :, 1] + 16.0
    p0 = np.ascontiguousarray(p0.reshape(128, 648))

    return {
        "x_cb": np.ascontiguousarray(x_cbv),
        "x_strip": x_strip,
        "w_off_t": w_off_t,
        "w_dcn_t": w_dcn_t,
        "gamma2": gamma2,
        "beta2": beta2,
        "p0": p0,
    }


def assemble_output(results):
    out = np.zeros((N, O, H, W), np.float32)
    for core in range(NCORES):
        n, half = core // 2, core % 2
        y = np.asarray(results[core]["y_out"], np.float32)
        out[n, :, half * ROWS : (half + 1) * ROWS, :] = y.reshape(O, ROWS, W)
    return out


_COMPILED = {}


def kernel(x, w_off, b_off, w_dcn, gamma, beta):
    from concourse import bass_utils

    if "nc" not in _COMPILED:
        _COMPILED["nc"] = build_program(NCORES)
    nc = _COMPILED["nc"]
    in_maps = [
        make_core_inputs(x, w_off, b_off, w_dcn, gamma, beta, core)
        for core in range(NCORES)
    ]
    res = bass_utils.run_bass_kernel_spmd(nc, in_maps, core_ids=list(range(NCORES)))
    return assemble_output(res.results)

